# revision 19
# baseline (speedup 1.0000x reference)
"""Multi-head attention (B=2, N=4096, D=512, H=8) on 8 trn2 NeuronCores.

Sharding: core c handles batch b = c//4 and head-pair p = c%4 (heads 2p,
2p+1).  Each core projects its batch's Q/K/V against its pair's weight
columns, computes transposed attention scores sT = K_h @ Q_h^T per
128-key chunk, applies exp((1/8)*sT) split between the ACT engine
(table exp) and the DVE (Schraudolph int16 bit-trick + 7-stage
quadratic correction op, sigma ~0.2%), multiplies by an augmented V
(extra ones column, M=65) so the softmax denominators fall out of the
same matmul, normalizes on-device (reciprocal of the denominator row,
broadcast across partitions via a tiny K=2 matmul, fused into the
PSUM->SBUF move of the attention output), and applies Wo with both
heads contracted in a single 128-deep matmul per output chunk.

Schedule notes (all aimed at keeping the PE busy 100% of the time so
it holds its top p-state):
  - only K0/V0/Q0 are projected up front; all other K/V/Q projections
    are woven into the attention stream of qc0 (K(nk) at kc=4nk-4,
    V(nk) at 4nk-3, V transposes at 4nk-2), so the PE is never
    DMA-starved.
  - scores for both heads of a key-chunk share one PSUM tile and one
    exp instruction ([128,1024]), halving exp instruction overhead.
  - attn@V matmuls run 4 key-chunks behind the scores matmuls so the
    PE never head-of-line blocks on exp results.
  - the end-of-qc normalization + out-projection are deferred into the
    next q-chunk's scores stream (den_bc at kc=1, o normalize at kc=1,
    out-proj pieces at kc=3,5,7,9) so the PE queue never waits on the
    DVE at a q-chunk boundary.
Head 1's V chunks use a [1|v] layout (ones column first) and its
attn@V output lands at partitions 63:128, so the normalizing
tensor_tensor multiplies are partition-aligned for both heads.

Device layouts (host pre-arranges):
  xt{q,k,v}: [8, 128, 2048]    tile (nk): X^T 4 x [128, 512] dc-chunks
  w{q,k,v}:  [128, 512]        w[p, dc*128+c] = W[dc*128+p, off+c]
  b{q,k,v}:  [128, 1]          pair slice of bias
  wo:        [128, 512]        wo[p, mt*128+c] = Wo[off+p, mt*128+c]
Output per core:
  y: [4, 8, 128, 512] bf16     tile (mt, qc): y^T[mt*128:+128, qc*512:+512],
                               already normalized and summed over the
                               core's two heads.
Final host step: out[b] = (sum_p y_p).T + bo
"""

import numpy as np

_B, _N, _D, _H, _DK = 2, 4096, 512, 8, 64
_NCORES = 8

_LN2 = float(np.log(2.0))
_A_DVE = 0.125 * 1024.0 / _LN2
_B_DVE = 15360.0
_CORR_A = -1.4763417585548537
_CORR_Q2 = 0.22711289921196798
_CORR_C = 0.9424678640725361

_nc_cache = {}
_exp_corr_op = None


def _get_exp_corr_op():
    """Register (once) the custom DVE op: out = ((u+C0)^2*C1 + C2) * Src0
    with u = bitwise_or(bitwise_and(Src0, Src1), 1.0f) — Src1 carries the
    fp32 mantissa mask 0x007FFFFF as a full-width tensor ([P,1] broadcast
    Src1 hangs the DVE on this runtime)."""
    global _exp_corr_op
    if _exp_corr_op is not None:
        return _exp_corr_op
    from concourse import dve_ops
    from concourse.dve_spec import (
        AluOp,
        Bin,
        C0,
        C1,
        C2,
        One,
        Spec,
        Src0,
        Src1,
        lower,
        sq,
    )
    from concourse.dve_uop import DveOpSpec

    name = "EXP16_CORR_ANT"
    for op in dve_ops.OPS:
        if op.name == name:
            _exp_corr_op = op
            return op

    u = Bin(AluOp.BITWISE_OR, Bin(AluOp.BITWISE_AND, Src0, Src1), One)
    body = (sq(u + C0) * C1 + C2) * Src0

    def _ref(in0, in1, s0, s1, imm2):
        b = np.asarray(in0, np.float32).view(np.uint32)
        m = np.asarray(in1, np.float32).view(np.uint32)
        uu = ((b & m) | np.uint32(0x3F800000)).view(np.float32)
        return ((uu + s0) ** 2 * s1 + imm2) * in0

    spec = Spec(body=body, reference=_ref)
    sha = {
        ver: DveOpSpec(name=name, uops=lower(spec, ver=ver)).sha(ver)
        for ver in ("v3", "v4")
    }
    op = dve_ops.DveOp(name, spec, subdim=False, uops_sha=sha)
    idx = len(dve_ops.OPS)
    dve_ops.OPS.append(op)
    dve_ops.CUSTOM_DVE_SPECS[name] = spec
    dve_ops._SUB_OPCODE_FOR_NAME[name] = dve_ops._CUSTOM_DVE_ROW_BASE + idx
    _exp_corr_op = op
    return op


def _build(
    n=_N,
    zero_bias=False,
    dve_split=True,
    mask16=False,
    pend_depth=4,
    dve_mod_steady=(2, 4, 6),  # kc % 8 in set -> DVE exp (12 of 32)
    dve_mod_qc0=(3,),  # kc % 4 in set -> DVE exp (8 of 32)
):
    import concourse.mybir as mybir
    import concourse.tile as tile
    from concourse import bacc
    from concourse.masks import make_identity

    f32 = mybir.dt.float32
    f32r = mybir.dt.float32r
    i16 = mybir.dt.int16
    i32 = mybir.dt.int32
    bf16 = mybir.dt.float16
    Exp = mybir.ActivationFunctionType.Exp
    Copy = mybir.ActivationFunctionType.Copy
    NKC = n // 128  # key chunks of 128 (PSUM partitions of sT)
    NQC = n // 512  # q chunks of 512

    corr_op = _get_exp_corr_op() if dve_split else None

    nc = bacc.Bacc(
        "TRN2", target_bir_lowering=False, debug=False, num_devices=_NCORES
    )

    xt = {
        t: nc.dram_tensor(f"xt{t}", [NQC, 128, 2048], bf16, kind="ExternalInput").ap()
        for t in "qkv"
    }
    w = {
        t: nc.dram_tensor(f"w{t}", [128, 512], bf16, kind="ExternalInput").ap()
        for t in "qkv"
    }
    bvec = {
        t: nc.dram_tensor(f"b{t}", [128, 1], f32, kind="ExternalInput").ap()
        for t in "qkv"
    }
    wo = nc.dram_tensor("wo", [128, 512], bf16, kind="ExternalInput").ap()
    ones2_in = nc.dram_tensor("ones2", [33, 128], bf16, kind="ExternalInput").ap()
    y_out = nc.dram_tensor(
        "y", [4, NQC, 128, 512], bf16, kind="ExternalOutput"
    ).ap()

    def dve_kc(qc, kc):
        if not dve_split:
            return False
        if qc == 0:
            return (kc % 4) in dve_mod_qc0
        return (kc % 8) in dve_mod_steady

    with tile.TileContext(nc) as tc:
        with (
            tc.tile_pool(name="consts", bufs=1) as consts,
            tc.tile_pool(name="xtp", bufs=6) as xtp,
            tc.tile_pool(name="persist", bufs=1) as persist,
            tc.tile_pool(name="ep", bufs=6) as ep,
            tc.tile_pool(name="ysbp", bufs=3) as ysbp,
            tc.tile_pool(name="dbp", bufs=2) as dbp,
            tc.tile_pool(name="psA", bufs=3, space="PSUM") as psA,
            tc.tile_pool(name="psB", bufs=1, space="PSUM") as psB,
        ):
            wsb, bsb = {}, {}
            for t in "qkv":
                wsb[t] = consts.tile([128, 512], bf16, name=f"w{t}sb", tag=f"w{t}sb")
            wosb = consts.tile([128, 512], bf16, name="wosb", tag="wosb")
            # k-path first on the sync queue so the first projection can start
            # as early as possible; v/q/wo ride the scalar HWDGE queue.
            nc.sync.dma_start(out=wsb["k"], in_=w["k"])
            nc.scalar.dma_start(out=wsb["v"], in_=w["v"])
            nc.scalar.dma_start(out=wsb["q"], in_=w["q"])
            nc.scalar.dma_start(out=wosb, in_=wo)
            if not zero_bias:
                for t in "qkv":
                    bsb[t] = consts.tile([128, 1], f32, name=f"b{t}sb", tag=f"b{t}sb")
                    nc.sync.dma_start(out=bsb[t], in_=bvec[t])
            ident = consts.tile([128, 128], bf16, name="ident")
            make_identity(nc, ident)
            if dve_split:
                mdt = bf16 if mask16 else f32
                mask_t = consts.tile([128, 1024], mdt, name="mmask", tag="mmask")
                if mask16:
                    nc.gpsimd.memset(mask_t.bitcast(i16), 0x007F)
                else:
                    nc.gpsimd.memset(mask_t.bitcast(i32), 0x007FFFFF)
            # ones2: stationary for the denominator partition-broadcast
            # matmul: den_bc[j, :] = den2r[0 if j < 64 else 32, :].  K=33
            # with zero rows 1..31 because engine APs need 32-aligned
            # partition bases (so h1's reciprocal lands at partition 32).
            ones2 = consts.tile([33, 128], bf16, name="ones2", tag="ones2")
            nc.scalar.dma_start(out=ones2, in_=ones2_in)

            qt_t = [
                persist.tile([128, 512], bf16, name=f"qt{i}", tag=f"qt{i}")
                for i in range(NQC)
            ]
            kt_t = [
                persist.tile([128, 512], bf16, name=f"kt{i}", tag=f"kt{i}")
                for i in range(NQC)
            ]
            vt_t = [
                persist.tile([128, 512], bf16, name=f"vt{i}", tag=f"vt{i}")
                for i in range(NQC)
            ]
            # augmented V chunks: [v|1] (ones col 64) for both heads; the
            # softmax denominator falls out of the attn@V matmul at
            # partition 64.
            vch = [
                [
                    persist.tile(
                        [128, 65], bf16, name=f"vch{h}_{c}", tag=f"vch{h}_{c}"
                    )
                    for c in range(NKC)
                ]
                for h in range(2)
            ]
            ot = [
                persist.tile([128, 512], bf16, name=f"ot{qc}", tag=f"ot{qc}")
                for qc in range(NQC)
            ]
            den2r_t = [
                persist.tile([33, 512], bf16, name=f"dr{qc}", tag=f"dr{qc}")
                for qc in range(NQC)
            ]
            for qc in range(NQC):
                nc.gpsimd.memset(den2r_t[qc], 0.0)
            for c in range(NKC):
                nc.gpsimd.memset(vch[0][c][:, 64:65], 1.0)
                nc.gpsimd.memset(vch[1][c][:, 64:65], 1.0)

            def proj(t, nk, dst, dma_eng=None, copy_eng="act"):
                ppsum = psA.tile([128, 1024], f32, name=f"pp_{t}{nk}", tag="sblk")
                xtile = xtp.tile([128, 2048], bf16, name=f"x_{t}{nk}", tag="xt")
                eng = dma_eng or nc.sync
                eng.dma_start(out=xtile[:, 0:1024], in_=xt[t][nk][:, 0:1024])
                eng.dma_start(out=xtile[:, 1024:2048], in_=xt[t][nk][:, 1024:2048])
                pp = ppsum[:, 0:512]
                for dc in range(4):
                    nc.tensor.matmul(
                        pp,
                        wsb[t][:, dc * 128 : (dc + 1) * 128],
                        xtile[:, dc * 512 : (dc + 1) * 512],
                        start=(dc == 0),
                        stop=(dc == 3),
                    )
                if not zero_bias:
                    nc.vector.tensor_scalar_add(out=dst, in0=pp, scalar1=bsb[t])
                elif copy_eng == "act":
                    nc.scalar.activation(out=dst, in_=pp, func=Copy)
                else:
                    nc.vector.tensor_copy(out=dst, in_=pp)

            def vtrans(nk):
                # transpose the 4 key-chunks of V tile nk into per-head
                # augmented chunks
                pt = psA.tile([128, 1024], bf16, name=f"pt{nk}", tag="sblk")
                for j in range(4):
                    nc.tensor.transpose(
                        pt[:, j * 128 : (j + 1) * 128],
                        vt_t[nk][:, j * 128 : (j + 1) * 128],
                        ident,
                    )
                for j in range(4):
                    c = nk * 4 + j
                    nc.vector.tensor_copy(
                        out=vch[0][c][:, 0:64], in_=pt[:, j * 128 : j * 128 + 64]
                    )
                    nc.vector.tensor_copy(
                        out=vch[1][c][:, 0:64], in_=pt[:, j * 128 + 64 : j * 128 + 128]
                    )

            # ---- phase 1: K0 / V0 / Q0 only; the rest is woven into qc0 ----
            proj("k", 0, kt_t[0], dma_eng=nc.sync, copy_eng="act")
            proj("v", 0, vt_t[0], dma_eng=nc.scalar, copy_eng="act")
            vtrans(0)
            proj("q", 0, qt_t[0], dma_eng=nc.scalar, copy_eng="dve")

            def make_outproj(qc, o_ps, den2r):
                """Deferred normalization + out-projection pieces for qc,
                executed interleaved with qc+1's scores stream."""
                den_bc_box = {}

                def den_bc_piece():
                    den_bc = psA.tile([128, 1024], f32, name=f"dbc{qc}", tag="sblk")
                    nc.tensor.matmul(
                        den_bc[:, 0:512],
                        ones2,
                        den2r,
                        start=True,
                        stop=True,
                        skip_group_check=True,
                    )
                    # DVE ops may read only one PSUM operand, so stage the
                    # broadcast denominators in SBUF
                    dbc_sb = dbp.tile([128, 512], bf16, name=f"db{qc}", tag="db")
                    nc.scalar.activation(out=dbc_sb, in_=den_bc[:, 0:512], func=Copy)
                    den_bc_box["t"] = dbc_sb

                def norm_piece():
                    dbc_sb = den_bc_box["t"]
                    nc.vector.tensor_mul(
                        out=ot[qc][0:64, :],
                        in0=o_ps[0][0:64, :],
                        in1=dbc_sb[0:64, :],
                    )
                    nc.vector.tensor_mul(
                        out=ot[qc][64:128, :],
                        in0=o_ps[1][0:64, :],
                        in1=dbc_sb[64:128, :],
                    )

                def mt_piece(mt):
                    y_ps = psA.tile([128, 1024], f32, name=f"y{qc}_{mt}", tag="sblk")
                    nc.tensor.matmul(
                        y_ps[:, 0:512],
                        wosb[:, mt * 128 : (mt + 1) * 128],
                        ot[qc],
                        start=True,
                        stop=True,
                        skip_group_check=True,
                    )
                    y_sb = ysbp.tile([128, 512], bf16, name=f"ysb{qc}_{mt}", tag="ysb")
                    if mt % 2 == 0:
                        nc.scalar.activation(out=y_sb, in_=y_ps[:, 0:512], func=Copy)
                    else:
                        nc.vector.tensor_copy(out=y_sb, in_=y_ps[:, 0:512])
                    nc.gpsimd.dma_start(out=y_out[mt, qc], in_=y_sb)

                return {
                    1: [den_bc_piece, norm_piece],
                    3: [lambda: mt_piece(0)],
                    5: [lambda: mt_piece(1)],
                    7: [lambda: mt_piece(2)],
                    9: [lambda: mt_piece(3)],
                }

            # ---- phase 2: attention, everything else woven in ----
            deferred = {}
            for qc in range(NQC):
                weave = {}
                if qc == 0:
                    for nk in range(1, NQC):
                        weave.setdefault(4 * nk - 4, []).append(
                            lambda nk=nk: proj(
                                "k", nk, kt_t[nk], dma_eng=nc.sync, copy_eng="act"
                            )
                        )
                        weave.setdefault(4 * nk - 3, []).append(
                            lambda nk=nk: proj(
                                "v", nk, vt_t[nk], dma_eng=nc.scalar, copy_eng="act"
                            )
                        )
                        weave.setdefault(4 * nk - 2, []).append(
                            lambda nk=nk: vtrans(nk)
                        )
                    weave.setdefault(14, []).append(
                        lambda: proj("q", 1, qt_t[1], dma_eng=nc.scalar, copy_eng="dve")
                    )
                    weave.setdefault(18, []).append(
                        lambda: proj("q", 2, qt_t[2], dma_eng=nc.scalar, copy_eng="dve")
                    )
                elif qc + 2 < NQC:
                    weave.setdefault(16, []).append(
                        lambda qc=qc: proj(
                            "q", qc + 2, qt_t[qc + 2],
                            dma_eng=nc.scalar, copy_eng="act",
                        )
                    )

                o_ps = {
                    0: psB.tile([128, 512], f32, name=f"o0_{qc}", tag="oy0"),
                    1: psB.tile([128, 512], f32, name=f"o1_{qc}", tag="oy1"),
                }

                def emit_o(blk, qc=qc, o_ps=o_ps):
                    kc, e_sb = blk
                    nc.tensor.matmul(
                        o_ps[0][0:65, :],
                        vch[0][kc],
                        e_sb[:, 0:512],
                        start=(kc == 0),
                        stop=(kc == NKC - 1),
                        skip_group_check=True,
                    )
                    nc.tensor.matmul(
                        o_ps[1][0:65, :],
                        vch[1][kc],
                        e_sb[:, 512:1024],
                        start=(kc == 0),
                        stop=(kc == NKC - 1),
                        skip_group_check=True,
                    )

                pend = []
                for kc in range(NKC):
                    for fn in weave.get(kc, []):
                        fn()
                    for fn in deferred.get(kc, []):
                        fn()
                    s_ps = psA.tile([128, 1024], f32, name=f"s_{qc}_{kc}", tag="sblk")
                    for h in range(2):
                        hp = slice(h * 64, (h + 1) * 64)
                        nc.tensor.matmul(
                            s_ps[:, h * 512 : (h + 1) * 512],
                            kt_t[kc // 4][hp, (kc % 4) * 128 : (kc % 4 + 1) * 128],
                            qt_t[qc][hp, :],
                            start=True,
                            stop=True,
                            skip_group_check=True,
                        )
                    e_sb = ep.tile([128, 1024], bf16, name=f"e_{qc}_{kc}", tag="e")
                    if dve_kc(qc, kc):
                        nc.vector.tensor_scalar(
                            out=e_sb.bitcast(i16),
                            in0=s_ps,
                            scalar1=_A_DVE,
                            scalar2=_B_DVE,
                            op0=mybir.AluOpType.mult,
                            op1=mybir.AluOpType.add,
                        )
                        nc.vector._custom_dve(
                            corr_op,
                            out=e_sb,
                            in0=e_sb,
                            in1=mask_t,
                            s0=_CORR_A,
                            s1=_CORR_Q2,
                            imm2=_CORR_C,
                        )
                    else:
                        nc.scalar.activation(e_sb, s_ps, Exp, scale=0.125)
                    pend.append((kc, e_sb))
                    if len(pend) > pend_depth:
                        emit_o(pend.pop(0))
                for blk in pend:
                    emit_o(blk)

                # softmax denominators -> reciprocals (den at partition 64;
                # h0 lands at den2r partition 0, h1 at partition 32)
                den2r = den2r_t[qc]
                with nc.allow_low_precision(reason="softmax denom broadcast"):
                    nc.vector.reciprocal(out=den2r[0:1, :], in_=o_ps[0][64:65, :])
                    nc.vector.reciprocal(out=den2r[32:33, :], in_=o_ps[1][64:65, :])
                deferred = make_outproj(qc, o_ps, den2r)

            # drain qc7's normalization + out-projection
            for kc in sorted(deferred):
                for fn in deferred[kc]:
                    fn()

    nc.finalize()
    return nc


def get_nc(n=_N, zero_bias=False, dve_split=True, **kw):
    key = (n, zero_bias, dve_split, tuple(sorted(kw.items())))
    if key not in _nc_cache:
        _nc_cache[key] = _build(n, zero_bias, dve_split, **kw)
    return _nc_cache[key]


def make_in_maps(Q, K, V, Wq, bq, Wk, bk, Wv, bv, Wo, bo, n=_N):
    """Per-core input dicts (host-side sharding / layout prep)."""
    bf = np.float16
    nqc = n // 512
    xts = {}
    for b in range(_B):
        d = {}
        for t, X in (("q", Q), ("k", K), ("v", V)):
            xt = X[b][:n].T.astype(bf)  # [512, n]
            d[f"xt{t}"] = np.ascontiguousarray(
                xt.reshape(4, 128, nqc, 512).transpose(2, 1, 0, 3).reshape(nqc, 128, 2048)
            )
        xts[b] = d
    in_maps = []
    for c in range(_NCORES):
        b, p = divmod(c, 4)
        off = p * 128
        m = dict(xts[b])
        for t, W, bias in (("q", Wq, bq), ("k", Wk, bk), ("v", Wv, bv)):
            m[f"w{t}"] = np.ascontiguousarray(
                W[:, off : off + 128]
                .reshape(4, 128, 128)
                .transpose(1, 0, 2)
                .reshape(128, 512)
                .astype(bf)
            )
            m[f"b{t}"] = np.ascontiguousarray(bias[off : off + 128].reshape(128, 1))
        m["wo"] = np.ascontiguousarray(Wo[off : off + 128].astype(bf))
        o2 = np.zeros((33, 128), np.float16)
        o2[0, 0:64] = 1.0
        o2[32, 64:128] = 1.0
        m["ones2"] = o2
        in_maps.append(m)
    return in_maps


def assemble(results, bo, n=_N):
    """Cross-core reduction: sum the (already normalized) per-head-pair
    partial outputs, add output bias, restore [B, N, D] layout."""
    out = np.empty((_B, n, _D), np.float32)
    for b in range(_B):
        acc = None
        for p in range(4):
            # y [4, nqc, 128, 512] -> [512, n]
            y = (
                results[4 * b + p]["y"]
                .astype(np.float32)
                .transpose(0, 2, 1, 3)
                .reshape(_D, n)
            )
            acc = y if acc is None else acc + y
        out[b] = acc.T + bo
    return out


def kernel(Q, K, V, Wq, bq, Wk, bk, Wv, bv, Wo, bo):
    from concourse import bass_utils

    args = [np.asarray(a, np.float32) for a in (Q, K, V, Wq, bq, Wk, bk, Wv, bv, Wo, bo)]
    Q, K, V, Wq, bq, Wk, bk, Wv, bv, Wo, bo = args
    zb = not (np.any(bq) or np.any(bk) or np.any(bv))
    nc = get_nc(zero_bias=zb)
    in_maps = make_in_maps(Q, K, V, Wq, bq, Wk, bk, Wv, bv, Wo, bo)
    res = bass_utils.run_bass_kernel_spmd(
        nc, in_maps, core_ids=list(range(_NCORES))
    )
    return assemble(res.results, bo)


# revision 24
# speedup vs baseline: 1.0580x; 1.0580x over previous
"""Multi-head attention (B=2, N=4096, D=512, H=8) on 8 trn2 NeuronCores.

Sharding: core c handles batch b = c//4 and head-pair p = c%4 (heads 2p,
2p+1).  Each core projects its batch's Q/K/V against its pair's weight
columns, computes transposed attention scores sT = K_h @ Q_h^T per
128-key chunk, applies exp((1/8)*sT) split between the ACT engine
(table exp) and the DVE (Schraudolph int16 bit-trick + 7-stage
quadratic correction op, sigma ~0.2%), multiplies by an augmented V
(extra ones column, M=65) so the softmax denominators fall out of the
same matmul, normalizes on-device (reciprocal of the denominator row,
broadcast across partitions via a tiny K=2 matmul, fused into the
PSUM->SBUF move of the attention output), and applies Wo with both
heads contracted in a single 128-deep matmul per output chunk.

Schedule notes (all aimed at keeping the PE busy 100% of the time so
it holds its top p-state):
  - only K0/V0/Q0 are projected up front; all other K/V/Q projections
    are woven into the attention stream of qc0 (K(nk) at kc=4nk-4,
    V(nk) at 4nk-3, V transposes at 4nk-2), so the PE is never
    DMA-starved.
  - scores for both heads of a key-chunk share one PSUM tile and one
    exp instruction ([128,1024]), halving exp instruction overhead.
  - attn@V matmuls run 4 key-chunks behind the scores matmuls so the
    PE never head-of-line blocks on exp results.
  - the end-of-qc normalization + out-projection are deferred into the
    next q-chunk's scores stream (den_bc at kc=1, o normalize at kc=1,
    out-proj pieces at kc=3,5,7,9) so the PE queue never waits on the
    DVE at a q-chunk boundary.
Head 1's V chunks use a [1|v] layout (ones column first) and its
attn@V output lands at partitions 63:128, so the normalizing
tensor_tensor multiplies are partition-aligned for both heads.

Device layouts (host pre-arranges):
  xt{q,k,v}: [8, 128, 2048]    tile (nk): X^T 4 x [128, 512] dc-chunks
  w{q,k,v}:  [128, 512]        w[p, dc*128+c] = W[dc*128+p, off+c]
  b{q,k,v}:  [128, 1]          pair slice of bias
  wo:        [128, 512]        wo[p, mt*128+c] = Wo[off+p, mt*128+c]
Output per core:
  y: [4, 8, 128, 512] bf16     tile (mt, qc): y^T[mt*128:+128, qc*512:+512],
                               already normalized and summed over the
                               core's two heads.
Final host step: out[b] = (sum_p y_p).T + bo
"""

import numpy as np

_B, _N, _D, _H, _DK = 2, 4096, 512, 8, 64
_NCORES = 8

_LN2 = float(np.log(2.0))
_A_DVE = 0.125 * 1024.0 / _LN2
_B_DVE = 15360.0
_CORR_A = -1.4763417585548537
_CORR_Q2 = 0.22711289921196798
_CORR_C = 0.9424678640725361

_nc_cache = {}
_exp_corr_op = None


def _get_exp_corr_op():
    """Register (once) the custom DVE op: out = ((u+C0)^2*C1 + C2) * Src0
    with u = bitwise_or(bitwise_and(Src0, Src1), 1.0f) — Src1 carries the
    fp32 mantissa mask 0x007FFFFF as a full-width tensor ([P,1] broadcast
    Src1 hangs the DVE on this runtime)."""
    global _exp_corr_op
    if _exp_corr_op is not None:
        return _exp_corr_op
    from concourse import dve_ops
    from concourse.dve_spec import (
        AluOp,
        Bin,
        C0,
        C1,
        C2,
        One,
        Spec,
        Src0,
        Src1,
        lower,
        sq,
    )
    from concourse.dve_uop import DveOpSpec

    name = "EXP16_CORR_ANT"
    for op in dve_ops.OPS:
        if op.name == name:
            _exp_corr_op = op
            return op

    u = Bin(AluOp.BITWISE_OR, Bin(AluOp.BITWISE_AND, Src0, Src1), One)
    body = (sq(u + C0) * C1 + C2) * Src0

    def _ref(in0, in1, s0, s1, imm2):
        b = np.asarray(in0, np.float32).view(np.uint32)
        m = np.asarray(in1, np.float32).view(np.uint32)
        uu = ((b & m) | np.uint32(0x3F800000)).view(np.float32)
        return ((uu + s0) ** 2 * s1 + imm2) * in0

    spec = Spec(body=body, reference=_ref)
    sha = {
        ver: DveOpSpec(name=name, uops=lower(spec, ver=ver)).sha(ver)
        for ver in ("v3", "v4")
    }
    op = dve_ops.DveOp(name, spec, subdim=False, uops_sha=sha)
    idx = len(dve_ops.OPS)
    dve_ops.OPS.append(op)
    dve_ops.CUSTOM_DVE_SPECS[name] = spec
    dve_ops._SUB_OPCODE_FOR_NAME[name] = dve_ops._CUSTOM_DVE_ROW_BASE + idx
    _exp_corr_op = op
    return op


def _build(
    n=_N,
    zero_bias=False,
    dve_split=True,
    mask16=False,
    pend_depth=6,
    # DVE exp key-chunks: early/middle kc only, so the DVE queue is fully
    # drained by the end of every q-chunk (the end-of-qc o_ps copies and
    # reciprocals must not sit behind exp backlog — the PE's in-order
    # queue would stall on them at the boundary).
    dve_kcs_steady=(5, 8, 11, 14, 17, 20, 23, 26),
    dve_kcs_qc0=(3, 7, 11, 15, 19, 23),
):
    import concourse.mybir as mybir
    import concourse.tile as tile
    from concourse import bacc
    from concourse.masks import make_identity

    f32 = mybir.dt.float32
    f32r = mybir.dt.float32r
    i16 = mybir.dt.int16
    i32 = mybir.dt.int32
    bf16 = mybir.dt.float16
    Exp = mybir.ActivationFunctionType.Exp
    Copy = mybir.ActivationFunctionType.Copy
    NKC = n // 128  # key chunks of 128 (PSUM partitions of sT)
    NQC = n // 512  # q chunks of 512

    corr_op = _get_exp_corr_op() if dve_split else None

    nc = bacc.Bacc(
        "TRN2", target_bir_lowering=False, debug=False, num_devices=_NCORES
    )

    xt = {
        t: nc.dram_tensor(f"xt{t}", [NQC, 128, 2048], bf16, kind="ExternalInput").ap()
        for t in "qkv"
    }
    w = {
        t: nc.dram_tensor(f"w{t}", [128, 512], bf16, kind="ExternalInput").ap()
        for t in "qkv"
    }
    bvec = {
        t: nc.dram_tensor(f"b{t}", [128, 1], f32, kind="ExternalInput").ap()
        for t in "qkv"
    }
    wo = nc.dram_tensor("wo", [128, 512], bf16, kind="ExternalInput").ap()
    ones2_in = nc.dram_tensor("ones2", [33, 128], bf16, kind="ExternalInput").ap()
    y_out = nc.dram_tensor(
        "y", [4, NQC, 128, 512], bf16, kind="ExternalOutput"
    ).ap()

    _dve_steady = frozenset(dve_kcs_steady)
    _dve_qc0 = frozenset(dve_kcs_qc0)

    def dve_kc(qc, kc):
        if not dve_split:
            return False
        return kc in (_dve_qc0 if qc == 0 else _dve_steady)

    with tile.TileContext(nc) as tc:
        with (
            tc.tile_pool(name="consts", bufs=1) as consts,
            tc.tile_pool(name="xtp", bufs=6) as xtp,
            tc.tile_pool(name="persist", bufs=1) as persist,
            tc.tile_pool(name="ep", bufs=8) as ep,
            tc.tile_pool(name="ysbp", bufs=3) as ysbp,
            tc.tile_pool(name="psA", bufs=3, space="PSUM") as psA,
            tc.tile_pool(name="psB", bufs=1, space="PSUM") as psB,
        ):
            wsb, bsb = {}, {}
            for t in "qkv":
                wsb[t] = consts.tile([128, 512], bf16, name=f"w{t}sb", tag=f"w{t}sb")
            wosb = consts.tile([128, 512], bf16, name="wosb", tag="wosb")
            # k-path first on the sync queue so the first projection can start
            # as early as possible; v/q/wo ride the scalar HWDGE queue.
            nc.sync.dma_start(out=wsb["k"], in_=w["k"])
            nc.scalar.dma_start(out=wsb["v"], in_=w["v"])
            nc.scalar.dma_start(out=wsb["q"], in_=w["q"])
            nc.scalar.dma_start(out=wosb, in_=wo)
            if not zero_bias:
                for t in "qkv":
                    bsb[t] = consts.tile([128, 1], f32, name=f"b{t}sb", tag=f"b{t}sb")
                    nc.sync.dma_start(out=bsb[t], in_=bvec[t])
            ident = consts.tile([128, 128], bf16, name="ident")
            make_identity(nc, ident)
            if dve_split:
                mdt = bf16 if mask16 else f32
                mask_t = consts.tile([128, 1024], mdt, name="mmask", tag="mmask")
                if mask16:
                    nc.gpsimd.memset(mask_t.bitcast(i16), 0x007F)
                else:
                    nc.gpsimd.memset(mask_t.bitcast(i32), 0x007FFFFF)
            # ones2: stationary for the denominator partition-broadcast
            # matmul: den_bc[j, :] = den2r[0 if j < 64 else 32, :].  K=33
            # with zero rows 1..31 because engine APs need 32-aligned
            # partition bases (so h1's reciprocal lands at partition 32).
            ones2 = consts.tile([33, 128], bf16, name="ones2", tag="ones2")
            nc.scalar.dma_start(out=ones2, in_=ones2_in)

            qt_t = [
                persist.tile([128, 512], bf16, name=f"qt{i}", tag=f"qt{i}")
                for i in range(NQC)
            ]
            kt_t = [
                persist.tile([128, 512], bf16, name=f"kt{i}", tag=f"kt{i}")
                for i in range(NQC)
            ]
            vt_t = [
                persist.tile([128, 512], bf16, name=f"vt{i}", tag=f"vt{i}")
                for i in range(NQC)
            ]
            # augmented V chunks: [v|1] (ones col 64) for both heads; the
            # softmax denominator falls out of the attn@V matmul at
            # partition 64.
            vch = [
                [
                    persist.tile(
                        [128, 65], bf16, name=f"vch{h}_{c}", tag=f"vch{h}_{c}"
                    )
                    for c in range(NKC)
                ]
                for h in range(2)
            ]
            ot = [
                persist.tile([128, 512], bf16, name=f"ot{qc}", tag=f"ot{qc}")
                for qc in range(NQC)
            ]
            den2r_t = [
                persist.tile([33, 512], bf16, name=f"dr{qc}", tag=f"dr{qc}")
                for qc in range(NQC)
            ]
            for qc in range(NQC):
                nc.gpsimd.memset(den2r_t[qc], 0.0)
            for c in range(NKC):
                nc.gpsimd.memset(vch[0][c][:, 64:65], 1.0)
                nc.gpsimd.memset(vch[1][c][:, 64:65], 1.0)

            def proj(t, nk, dst, dma_eng=None, copy_eng="act"):
                ppsum = psA.tile([128, 1024], f32, name=f"pp_{t}{nk}", tag="sblk")
                xtile = xtp.tile([128, 2048], bf16, name=f"x_{t}{nk}", tag="xt")
                eng = dma_eng or nc.sync
                eng.dma_start(out=xtile[:, 0:1024], in_=xt[t][nk][:, 0:1024])
                eng.dma_start(out=xtile[:, 1024:2048], in_=xt[t][nk][:, 1024:2048])
                pp = ppsum[:, 0:512]
                for dc in range(4):
                    nc.tensor.matmul(
                        pp,
                        wsb[t][:, dc * 128 : (dc + 1) * 128],
                        xtile[:, dc * 512 : (dc + 1) * 512],
                        start=(dc == 0),
                        stop=(dc == 3),
                    )
                if not zero_bias:
                    nc.vector.tensor_scalar_add(out=dst, in0=pp, scalar1=bsb[t])
                elif copy_eng == "act":
                    nc.scalar.activation(out=dst, in_=pp, func=Copy)
                else:
                    nc.vector.tensor_copy(out=dst, in_=pp)

            def vtrans(nk):
                # transpose the 4 key-chunks of V tile nk into per-head
                # augmented chunks
                pt = psA.tile([128, 1024], bf16, name=f"pt{nk}", tag="sblk")
                for j in range(4):
                    nc.tensor.transpose(
                        pt[:, j * 128 : (j + 1) * 128],
                        vt_t[nk][:, j * 128 : (j + 1) * 128],
                        ident,
                    )
                for j in range(4):
                    c = nk * 4 + j
                    nc.vector.tensor_copy(
                        out=vch[0][c][:, 0:64], in_=pt[:, j * 128 : j * 128 + 64]
                    )
                    nc.vector.tensor_copy(
                        out=vch[1][c][:, 0:64], in_=pt[:, j * 128 + 64 : j * 128 + 128]
                    )

            # ---- phase 1: K0 / V0 / Q0 only; the rest is woven into qc0 ----
            proj("k", 0, kt_t[0], dma_eng=nc.sync, copy_eng="act")
            proj("v", 0, vt_t[0], dma_eng=nc.scalar, copy_eng="act")
            vtrans(0)
            proj("q", 0, qt_t[0], dma_eng=nc.scalar, copy_eng="dve")

            def make_outproj(qc, o_ps, den2r):
                """Deferred normalization + out-projection pieces for qc,
                executed interleaved with qc+1's scores stream.  ot[qc]
                already holds the raw attention output (copied at the end
                of qc so the o_ps PSUM banks free early); here we broadcast
                the reciprocal denominators, scale ot in place, and run the
                out-projection."""
                den_bc_box = {}

                def den_bc_piece():
                    den_bc = psA.tile([128, 1024], f32, name=f"dbc{qc}", tag="sblk")
                    nc.tensor.matmul(
                        den_bc[:, 0:512],
                        ones2,
                        den2r,
                        start=True,
                        stop=True,
                        skip_group_check=True,
                    )
                    den_bc_box["t"] = den_bc

                def norm_piece():
                    den_bc = den_bc_box["t"]
                    nc.vector.tensor_mul(
                        out=ot[qc][0:64, :],
                        in0=ot[qc][0:64, :],
                        in1=den_bc[0:64, 0:512],
                    )
                    nc.vector.tensor_mul(
                        out=ot[qc][64:128, :],
                        in0=ot[qc][64:128, :],
                        in1=den_bc[64:128, 0:512],
                    )

                def mt_piece(mt):
                    y_ps = psA.tile([128, 1024], f32, name=f"y{qc}_{mt}", tag="sblk")
                    nc.tensor.matmul(
                        y_ps[:, 0:512],
                        wosb[:, mt * 128 : (mt + 1) * 128],
                        ot[qc],
                        start=True,
                        stop=True,
                        skip_group_check=True,
                    )
                    y_sb = ysbp.tile([128, 512], bf16, name=f"ysb{qc}_{mt}", tag="ysb")
                    if mt % 2 == 0:
                        nc.scalar.activation(out=y_sb, in_=y_ps[:, 0:512], func=Copy)
                    else:
                        nc.vector.tensor_copy(out=y_sb, in_=y_ps[:, 0:512])
                    nc.gpsimd.dma_start(out=y_out[mt, qc], in_=y_sb)

                return {
                    2: [den_bc_piece],
                    3: [norm_piece],
                    5: [lambda: mt_piece(0)],
                    7: [lambda: mt_piece(1)],
                    9: [lambda: mt_piece(2)],
                    11: [lambda: mt_piece(3)],
                }

            # ---- phase 2: attention, everything else woven in ----
            deferred = {}
            for qc in range(NQC):
                weave = {}
                if qc == 0:
                    for nk in range(1, NQC):
                        weave.setdefault(4 * nk - 4, []).append(
                            lambda nk=nk: proj(
                                "k", nk, kt_t[nk], dma_eng=nc.sync, copy_eng="act"
                            )
                        )
                        weave.setdefault(4 * nk - 3, []).append(
                            lambda nk=nk: proj(
                                "v", nk, vt_t[nk], dma_eng=nc.scalar, copy_eng="act"
                            )
                        )
                        weave.setdefault(4 * nk - 2, []).append(
                            lambda nk=nk: vtrans(nk)
                        )
                    weave.setdefault(14, []).append(
                        lambda: proj("q", 1, qt_t[1], dma_eng=nc.scalar, copy_eng="dve")
                    )
                    weave.setdefault(18, []).append(
                        lambda: proj("q", 2, qt_t[2], dma_eng=nc.scalar, copy_eng="dve")
                    )
                elif qc + 2 < NQC:
                    weave.setdefault(16, []).append(
                        lambda qc=qc: proj(
                            "q", qc + 2, qt_t[qc + 2],
                            dma_eng=nc.scalar, copy_eng="act",
                        )
                    )

                o_ps = {
                    0: psB.tile([128, 512], f32, name=f"o0_{qc}", tag="oy0"),
                    1: psB.tile([128, 512], f32, name=f"o1_{qc}", tag="oy1"),
                }

                def emit_o(blk, qc=qc, o_ps=o_ps):
                    kc, e_sb = blk
                    nc.tensor.matmul(
                        o_ps[0][0:65, :],
                        vch[0][kc],
                        e_sb[:, 0:512],
                        start=(kc == 0),
                        stop=(kc == NKC - 1),
                        skip_group_check=True,
                    )
                    nc.tensor.matmul(
                        o_ps[1][0:65, :],
                        vch[1][kc],
                        e_sb[:, 512:1024],
                        start=(kc == 0),
                        stop=(kc == NKC - 1),
                        skip_group_check=True,
                    )

                pend = []
                for kc in range(NKC):
                    for fn in weave.get(kc, []):
                        fn()
                    for fn in deferred.get(kc, []):
                        fn()
                    s_ps = psA.tile([128, 1024], f32, name=f"s_{qc}_{kc}", tag="sblk")
                    for h in range(2):
                        hp = slice(h * 64, (h + 1) * 64)
                        nc.tensor.matmul(
                            s_ps[:, h * 512 : (h + 1) * 512],
                            kt_t[kc // 4][hp, (kc % 4) * 128 : (kc % 4 + 1) * 128],
                            qt_t[qc][hp, :],
                            start=True,
                            stop=True,
                            skip_group_check=True,
                        )
                    e_sb = ep.tile([128, 1024], bf16, name=f"e_{qc}_{kc}", tag="e")
                    if dve_kc(qc, kc):
                        nc.vector.tensor_scalar(
                            out=e_sb.bitcast(i16),
                            in0=s_ps,
                            scalar1=_A_DVE,
                            scalar2=_B_DVE,
                            op0=mybir.AluOpType.mult,
                            op1=mybir.AluOpType.add,
                        )
                        nc.vector._custom_dve(
                            corr_op,
                            out=e_sb,
                            in0=e_sb,
                            in1=mask_t,
                            s0=_CORR_A,
                            s1=_CORR_Q2,
                            imm2=_CORR_C,
                        )
                    else:
                        nc.scalar.activation(e_sb, s_ps, Exp, scale=0.125)
                    pend.append((kc, e_sb))
                    if len(pend) > pend_depth:
                        emit_o(pend.pop(0))
                for blk in pend:
                    emit_o(blk)

                # park the raw attention output in SBUF immediately (frees
                # the o_ps PSUM banks for qc+1), then take the denominator
                # reciprocals (den at partition 64; h0 -> den2r partition 0,
                # h1 -> partition 32)
                nc.vector.tensor_copy(out=ot[qc][0:64, :], in_=o_ps[0][0:64, :])
                nc.vector.tensor_copy(out=ot[qc][64:128, :], in_=o_ps[1][0:64, :])
                den2r = den2r_t[qc]
                with nc.allow_low_precision(reason="softmax denom broadcast"):
                    nc.vector.reciprocal(out=den2r[0:1, :], in_=o_ps[0][64:65, :])
                    nc.vector.reciprocal(out=den2r[32:33, :], in_=o_ps[1][64:65, :])
                deferred = make_outproj(qc, o_ps, den2r)

            # drain qc7's normalization + out-projection
            for kc in sorted(deferred):
                for fn in deferred[kc]:
                    fn()

    nc.finalize()
    return nc


def get_nc(n=_N, zero_bias=False, dve_split=True, **kw):
    key = (n, zero_bias, dve_split, tuple(sorted(kw.items())))
    if key not in _nc_cache:
        _nc_cache[key] = _build(n, zero_bias, dve_split, **kw)
    return _nc_cache[key]


def make_in_maps(Q, K, V, Wq, bq, Wk, bk, Wv, bv, Wo, bo, n=_N):
    """Per-core input dicts (host-side sharding / layout prep)."""
    bf = np.float16
    nqc = n // 512
    xts = {}
    for b in range(_B):
        d = {}
        for t, X in (("q", Q), ("k", K), ("v", V)):
            xt = X[b][:n].T.astype(bf)  # [512, n]
            d[f"xt{t}"] = np.ascontiguousarray(
                xt.reshape(4, 128, nqc, 512).transpose(2, 1, 0, 3).reshape(nqc, 128, 2048)
            )
        xts[b] = d
    in_maps = []
    for c in range(_NCORES):
        b, p = divmod(c, 4)
        off = p * 128
        m = dict(xts[b])
        for t, W, bias in (("q", Wq, bq), ("k", Wk, bk), ("v", Wv, bv)):
            m[f"w{t}"] = np.ascontiguousarray(
                W[:, off : off + 128]
                .reshape(4, 128, 128)
                .transpose(1, 0, 2)
                .reshape(128, 512)
                .astype(bf)
            )
            m[f"b{t}"] = np.ascontiguousarray(bias[off : off + 128].reshape(128, 1))
        m["wo"] = np.ascontiguousarray(Wo[off : off + 128].astype(bf))
        o2 = np.zeros((33, 128), np.float16)
        o2[0, 0:64] = 1.0
        o2[32, 64:128] = 1.0
        m["ones2"] = o2
        in_maps.append(m)
    return in_maps


def assemble(results, bo, n=_N):
    """Cross-core reduction: sum the (already normalized) per-head-pair
    partial outputs, add output bias, restore [B, N, D] layout."""
    out = np.empty((_B, n, _D), np.float32)
    for b in range(_B):
        acc = None
        for p in range(4):
            # y [4, nqc, 128, 512] -> [512, n]
            y = (
                results[4 * b + p]["y"]
                .astype(np.float32)
                .transpose(0, 2, 1, 3)
                .reshape(_D, n)
            )
            acc = y if acc is None else acc + y
        out[b] = acc.T + bo
    return out


def kernel(Q, K, V, Wq, bq, Wk, bk, Wv, bv, Wo, bo):
    from concourse import bass_utils

    args = [np.asarray(a, np.float32) for a in (Q, K, V, Wq, bq, Wk, bk, Wv, bv, Wo, bo)]
    Q, K, V, Wq, bq, Wk, bk, Wv, bv, Wo, bo = args
    zb = not (np.any(bq) or np.any(bk) or np.any(bv))
    nc = get_nc(zero_bias=zb)
    in_maps = make_in_maps(Q, K, V, Wq, bq, Wk, bk, Wv, bv, Wo, bo)
    res = bass_utils.run_bass_kernel_spmd(
        nc, in_maps, core_ids=list(range(_NCORES))
    )
    return assemble(res.results, bo)


# revision 27
# speedup vs baseline: 1.0791x; 1.0200x over previous
"""Multi-head attention (B=2, N=4096, D=512, H=8) on 8 trn2 NeuronCores.

Sharding: core c handles batch b = c//4 and head-pair p = c%4 (heads 2p,
2p+1).  Each core projects its batch's Q/K/V against its pair's weight
columns, computes transposed attention scores sT = K_h @ Q_h^T per
128-key chunk, applies exp((1/8)*sT) split between the ACT engine
(table exp) and the DVE (Schraudolph int16 bit-trick + 7-stage
quadratic correction op, sigma ~0.2%), multiplies by an augmented V
(extra ones column, M=65) so the softmax denominators fall out of the
same matmul, normalizes on-device (reciprocal of the denominator row,
broadcast across partitions via a tiny K=2 matmul, fused into the
PSUM->SBUF move of the attention output), and applies Wo with both
heads contracted in a single 128-deep matmul per output chunk.

Schedule notes (all aimed at keeping the PE busy 100% of the time so
it holds its top p-state):
  - only K0/V0/Q0 are projected up front; all other K/V/Q projections
    are woven into the attention stream of qc0 (K(nk) at kc=4nk-4,
    V(nk) at 4nk-3, V transposes at 4nk-2), so the PE is never
    DMA-starved.
  - scores for both heads of a key-chunk share one PSUM tile and one
    exp instruction ([128,1024]), halving exp instruction overhead.
  - attn@V matmuls run 4 key-chunks behind the scores matmuls so the
    PE never head-of-line blocks on exp results.
  - the end-of-qc normalization + out-projection are deferred into the
    next q-chunk's scores stream (den_bc at kc=1, o normalize at kc=1,
    out-proj pieces at kc=3,5,7,9) so the PE queue never waits on the
    DVE at a q-chunk boundary.
Head 1's V chunks use a [1|v] layout (ones column first) and its
attn@V output lands at partitions 63:128, so the normalizing
tensor_tensor multiplies are partition-aligned for both heads.

Device layouts (host pre-arranges):
  xt{q,k,v}: [8, 128, 2048]    tile (nk): X^T 4 x [128, 512] dc-chunks
  w{q,k,v}:  [128, 512]        w[p, dc*128+c] = W[dc*128+p, off+c]
  b{q,k,v}:  [128, 1]          pair slice of bias
  wo:        [128, 512]        wo[p, mt*128+c] = Wo[off+p, mt*128+c]
Output per core:
  y: [4, 8, 128, 512] bf16     tile (mt, qc): y^T[mt*128:+128, qc*512:+512],
                               already normalized and summed over the
                               core's two heads.
Final host step: out[b] = (sum_p y_p).T + bo
"""

import numpy as np

_B, _N, _D, _H, _DK = 2, 4096, 512, 8, 64
_NCORES = 8

_LN2 = float(np.log(2.0))
_A_DVE = 0.125 * 1024.0 / _LN2
_B_DVE = 15360.0
_CORR_A = -1.4763417585548537
_CORR_Q2 = 0.22711289921196798
_CORR_C = 0.9424678640725361

_nc_cache = {}
_exp_corr_op = None


def _get_exp_corr_op():
    """Register (once) the custom DVE op: out = ((u+C0)^2*C1 + C2) * Src0
    with u = bitwise_or(bitwise_and(Src0, Src1), 1.0f) — Src1 carries the
    fp32 mantissa mask 0x007FFFFF as a full-width tensor ([P,1] broadcast
    Src1 hangs the DVE on this runtime)."""
    global _exp_corr_op
    if _exp_corr_op is not None:
        return _exp_corr_op
    from concourse import dve_ops
    from concourse.dve_spec import (
        AluOp,
        Bin,
        C0,
        C1,
        C2,
        One,
        Spec,
        Src0,
        Src1,
        lower,
        sq,
    )
    from concourse.dve_uop import DveOpSpec

    name = "EXP16_CORR_ANT"
    for op in dve_ops.OPS:
        if op.name == name:
            _exp_corr_op = op
            return op

    u = Bin(AluOp.BITWISE_OR, Bin(AluOp.BITWISE_AND, Src0, Src1), One)
    body = (sq(u + C0) * C1 + C2) * Src0

    def _ref(in0, in1, s0, s1, imm2):
        b = np.asarray(in0, np.float32).view(np.uint32)
        m = np.asarray(in1, np.float32).view(np.uint32)
        uu = ((b & m) | np.uint32(0x3F800000)).view(np.float32)
        return ((uu + s0) ** 2 * s1 + imm2) * in0

    spec = Spec(body=body, reference=_ref)
    sha = {
        ver: DveOpSpec(name=name, uops=lower(spec, ver=ver)).sha(ver)
        for ver in ("v3", "v4")
    }
    op = dve_ops.DveOp(name, spec, subdim=False, uops_sha=sha)
    idx = len(dve_ops.OPS)
    dve_ops.OPS.append(op)
    dve_ops.CUSTOM_DVE_SPECS[name] = spec
    dve_ops._SUB_OPCODE_FOR_NAME[name] = dve_ops._CUSTOM_DVE_ROW_BASE + idx
    _exp_corr_op = op
    return op


def _build(
    n=_N,
    zero_bias=False,
    dve_split=True,
    mask16=False,
    pend_depth=7,
    # DVE exp key-chunks: middle kc only.  The first few kc of each qc
    # stay on ACT right after its boundary backlog clears (they feed the
    # first attn@V matmuls), and the last kc stay off the DVE so the
    # end-of-qc reciprocals/copies don't sit behind exp backlog — the
    # PE's in-order queue stalls on them at the boundary otherwise.
    dve_kcs_steady=(8, 11, 14, 17, 20, 23, 26, 29),
    dve_kcs_qc0=(7, 11, 15, 19, 23),
):
    import concourse.mybir as mybir
    import concourse.tile as tile
    from concourse import bacc
    from concourse.masks import make_identity

    f32 = mybir.dt.float32
    f32r = mybir.dt.float32r
    i16 = mybir.dt.int16
    i32 = mybir.dt.int32
    bf16 = mybir.dt.float16
    Exp = mybir.ActivationFunctionType.Exp
    Copy = mybir.ActivationFunctionType.Copy
    NKC = n // 128  # key chunks of 128 (PSUM partitions of sT)
    NQC = n // 512  # q chunks of 512

    corr_op = _get_exp_corr_op() if dve_split else None

    nc = bacc.Bacc(
        "TRN2", target_bir_lowering=False, debug=False, num_devices=_NCORES
    )

    xt = {
        t: nc.dram_tensor(f"xt{t}", [NQC, 128, 2048], bf16, kind="ExternalInput").ap()
        for t in "qkv"
    }
    w = {
        t: nc.dram_tensor(f"w{t}", [128, 512], bf16, kind="ExternalInput").ap()
        for t in "qkv"
    }
    bvec = {
        t: nc.dram_tensor(f"b{t}", [128, 1], f32, kind="ExternalInput").ap()
        for t in "qkv"
    }
    wo = nc.dram_tensor("wo", [128, 512], bf16, kind="ExternalInput").ap()
    ones2_in = nc.dram_tensor("ones2", [33, 128], bf16, kind="ExternalInput").ap()
    y_out = nc.dram_tensor(
        "y", [4, NQC, 128, 512], bf16, kind="ExternalOutput"
    ).ap()

    _dve_steady = frozenset(dve_kcs_steady)
    _dve_qc0 = frozenset(dve_kcs_qc0)

    def dve_kc(qc, kc):
        if not dve_split:
            return False
        return kc in (_dve_qc0 if qc == 0 else _dve_steady)

    with tile.TileContext(nc) as tc:
        with (
            tc.tile_pool(name="consts", bufs=1) as consts,
            tc.tile_pool(name="xtp", bufs=6) as xtp,
            tc.tile_pool(name="persist", bufs=1) as persist,
            tc.tile_pool(name="ep", bufs=9) as ep,
            tc.tile_pool(name="ysbp", bufs=3) as ysbp,
            tc.tile_pool(name="psA", bufs=3, space="PSUM") as psA,
            tc.tile_pool(name="psB", bufs=1, space="PSUM") as psB,
        ):
            wsb, bsb = {}, {}
            for t in "qkv":
                wsb[t] = consts.tile([128, 512], bf16, name=f"w{t}sb", tag=f"w{t}sb")
            wosb = consts.tile([128, 512], bf16, name="wosb", tag="wosb")
            # k-path first on the sync queue so the first projection can start
            # as early as possible; v/q/wo ride the scalar HWDGE queue.
            nc.sync.dma_start(out=wsb["k"], in_=w["k"])
            nc.scalar.dma_start(out=wsb["v"], in_=w["v"])
            nc.scalar.dma_start(out=wsb["q"], in_=w["q"])
            nc.scalar.dma_start(out=wosb, in_=wo)
            if not zero_bias:
                for t in "qkv":
                    bsb[t] = consts.tile([128, 1], f32, name=f"b{t}sb", tag=f"b{t}sb")
                    nc.sync.dma_start(out=bsb[t], in_=bvec[t])
            ident = consts.tile([128, 128], bf16, name="ident")
            make_identity(nc, ident)
            if dve_split:
                mdt = bf16 if mask16 else f32
                mask_t = consts.tile([128, 1024], mdt, name="mmask", tag="mmask")
                if mask16:
                    nc.gpsimd.memset(mask_t.bitcast(i16), 0x007F)
                else:
                    nc.gpsimd.memset(mask_t.bitcast(i32), 0x007FFFFF)
            # ones2: stationary for the denominator partition-broadcast
            # matmul: den_bc[j, :] = den2r[0 if j < 64 else 32, :].  K=33
            # with zero rows 1..31 because engine APs need 32-aligned
            # partition bases (so h1's reciprocal lands at partition 32).
            ones2 = consts.tile([33, 128], bf16, name="ones2", tag="ones2")
            nc.scalar.dma_start(out=ones2, in_=ones2_in)

            qt_t = [
                persist.tile([128, 512], bf16, name=f"qt{i}", tag=f"qt{i}")
                for i in range(NQC)
            ]
            kt_t = [
                persist.tile([128, 512], bf16, name=f"kt{i}", tag=f"kt{i}")
                for i in range(NQC)
            ]
            vt_t = [
                persist.tile([128, 512], bf16, name=f"vt{i}", tag=f"vt{i}")
                for i in range(NQC)
            ]
            # augmented V chunks: [v|1] (ones col 64) for both heads; the
            # softmax denominator falls out of the attn@V matmul at
            # partition 64.
            vch = [
                [
                    persist.tile(
                        [128, 65], bf16, name=f"vch{h}_{c}", tag=f"vch{h}_{c}"
                    )
                    for c in range(NKC)
                ]
                for h in range(2)
            ]
            ot = [
                persist.tile([128, 512], bf16, name=f"ot{qc}", tag=f"ot{qc}")
                for qc in range(NQC)
            ]
            den2r_t = [
                persist.tile([33, 512], bf16, name=f"dr{qc}", tag=f"dr{qc}")
                for qc in range(NQC)
            ]
            for qc in range(NQC):
                nc.gpsimd.memset(den2r_t[qc], 0.0)
            for c in range(NKC):
                nc.gpsimd.memset(vch[0][c][:, 64:65], 1.0)
                nc.gpsimd.memset(vch[1][c][:, 64:65], 1.0)

            def proj(t, nk, dst, dma_eng=None, copy_eng="act"):
                ppsum = psA.tile([128, 1024], f32, name=f"pp_{t}{nk}", tag="sblk")
                xtile = xtp.tile([128, 2048], bf16, name=f"x_{t}{nk}", tag="xt")
                eng = dma_eng or nc.sync
                eng.dma_start(out=xtile[:, 0:1024], in_=xt[t][nk][:, 0:1024])
                eng.dma_start(out=xtile[:, 1024:2048], in_=xt[t][nk][:, 1024:2048])
                pp = ppsum[:, 0:512]
                for dc in range(4):
                    nc.tensor.matmul(
                        pp,
                        wsb[t][:, dc * 128 : (dc + 1) * 128],
                        xtile[:, dc * 512 : (dc + 1) * 512],
                        start=(dc == 0),
                        stop=(dc == 3),
                    )
                if not zero_bias:
                    nc.vector.tensor_scalar_add(out=dst, in0=pp, scalar1=bsb[t])
                elif copy_eng == "act":
                    nc.scalar.activation(out=dst, in_=pp, func=Copy)
                else:
                    nc.vector.tensor_copy(out=dst, in_=pp)

            def vtrans(nk):
                # transpose the 4 key-chunks of V tile nk into per-head
                # augmented chunks
                pt = psA.tile([128, 1024], bf16, name=f"pt{nk}", tag="sblk")
                for j in range(4):
                    nc.tensor.transpose(
                        pt[:, j * 128 : (j + 1) * 128],
                        vt_t[nk][:, j * 128 : (j + 1) * 128],
                        ident,
                    )
                for j in range(4):
                    c = nk * 4 + j
                    nc.vector.tensor_copy(
                        out=vch[0][c][:, 0:64], in_=pt[:, j * 128 : j * 128 + 64]
                    )
                    nc.vector.tensor_copy(
                        out=vch[1][c][:, 0:64], in_=pt[:, j * 128 + 64 : j * 128 + 128]
                    )

            # ---- phase 1: K0 / V0 / Q0 only; the rest is woven into qc0 ----
            proj("k", 0, kt_t[0], dma_eng=nc.sync, copy_eng="act")
            proj("v", 0, vt_t[0], dma_eng=nc.scalar, copy_eng="act")
            vtrans(0)
            proj("q", 0, qt_t[0], dma_eng=nc.scalar, copy_eng="dve")

            def make_outproj(qc, o_ps, den2r):
                """Deferred normalization + out-projection pieces for qc,
                executed interleaved with qc+1's scores stream.  ot[qc]
                already holds the raw attention output (copied at the end
                of qc so the o_ps PSUM banks free early); here we broadcast
                the reciprocal denominators, scale ot in place, and run the
                out-projection."""
                den_bc_box = {}

                def den_bc_piece():
                    den_bc = psA.tile([128, 1024], f32, name=f"dbc{qc}", tag="sblk")
                    nc.tensor.matmul(
                        den_bc[:, 0:512],
                        ones2,
                        den2r,
                        start=True,
                        stop=True,
                        skip_group_check=True,
                    )
                    den_bc_box["t"] = den_bc

                def norm_piece():
                    den_bc = den_bc_box["t"]
                    nc.vector.tensor_mul(
                        out=ot[qc][0:64, :],
                        in0=ot[qc][0:64, :],
                        in1=den_bc[0:64, 0:512],
                    )
                    nc.vector.tensor_mul(
                        out=ot[qc][64:128, :],
                        in0=ot[qc][64:128, :],
                        in1=den_bc[64:128, 0:512],
                    )

                def mt_piece(mt):
                    y_ps = psA.tile([128, 1024], f32, name=f"y{qc}_{mt}", tag="sblk")
                    nc.tensor.matmul(
                        y_ps[:, 0:512],
                        wosb[:, mt * 128 : (mt + 1) * 128],
                        ot[qc],
                        start=True,
                        stop=True,
                        skip_group_check=True,
                    )
                    y_sb = ysbp.tile([128, 512], bf16, name=f"ysb{qc}_{mt}", tag="ysb")
                    if mt % 2 == 0:
                        nc.scalar.activation(out=y_sb, in_=y_ps[:, 0:512], func=Copy)
                    else:
                        nc.vector.tensor_copy(out=y_sb, in_=y_ps[:, 0:512])
                    nc.gpsimd.dma_start(out=y_out[mt, qc], in_=y_sb)

                return {
                    2: [den_bc_piece],
                    3: [norm_piece],
                    5: [lambda: mt_piece(0)],
                    7: [lambda: mt_piece(1)],
                    9: [lambda: mt_piece(2)],
                    11: [lambda: mt_piece(3)],
                }

            # ---- phase 2: attention, everything else woven in ----
            deferred = {}
            for qc in range(NQC):
                weave = {}
                if qc == 0:
                    for nk in range(1, NQC):
                        weave.setdefault(4 * nk - 4, []).append(
                            lambda nk=nk: proj(
                                "k", nk, kt_t[nk], dma_eng=nc.sync, copy_eng="act"
                            )
                        )
                        weave.setdefault(4 * nk - 3, []).append(
                            lambda nk=nk: proj(
                                "v", nk, vt_t[nk], dma_eng=nc.scalar, copy_eng="act"
                            )
                        )
                        weave.setdefault(4 * nk - 2, []).append(
                            lambda nk=nk: vtrans(nk)
                        )
                    weave.setdefault(14, []).append(
                        lambda: proj("q", 1, qt_t[1], dma_eng=nc.scalar, copy_eng="dve")
                    )
                    weave.setdefault(18, []).append(
                        lambda: proj("q", 2, qt_t[2], dma_eng=nc.scalar, copy_eng="dve")
                    )
                elif qc + 2 < NQC:
                    weave.setdefault(16, []).append(
                        lambda qc=qc: proj(
                            "q", qc + 2, qt_t[qc + 2],
                            dma_eng=nc.scalar, copy_eng="act",
                        )
                    )

                o_ps = {
                    0: psB.tile([128, 512], f32, name=f"o0_{qc}", tag="oy0"),
                    1: psB.tile([128, 512], f32, name=f"o1_{qc}", tag="oy1"),
                }

                def emit_o(blk, qc=qc, o_ps=o_ps):
                    kc, e_sb = blk
                    nc.tensor.matmul(
                        o_ps[0][0:65, :],
                        vch[0][kc],
                        e_sb[:, 0:512],
                        start=(kc == 0),
                        stop=(kc == NKC - 1),
                        skip_group_check=True,
                    )
                    nc.tensor.matmul(
                        o_ps[1][0:65, :],
                        vch[1][kc],
                        e_sb[:, 512:1024],
                        start=(kc == 0),
                        stop=(kc == NKC - 1),
                        skip_group_check=True,
                    )

                pend = []
                for kc in range(NKC):
                    for fn in weave.get(kc, []):
                        fn()
                    for fn in deferred.get(kc, []):
                        fn()
                    s_ps = psA.tile([128, 1024], f32, name=f"s_{qc}_{kc}", tag="sblk")
                    for h in range(2):
                        hp = slice(h * 64, (h + 1) * 64)
                        nc.tensor.matmul(
                            s_ps[:, h * 512 : (h + 1) * 512],
                            kt_t[kc // 4][hp, (kc % 4) * 128 : (kc % 4 + 1) * 128],
                            qt_t[qc][hp, :],
                            start=True,
                            stop=True,
                            skip_group_check=True,
                        )
                    e_sb = ep.tile([128, 1024], bf16, name=f"e_{qc}_{kc}", tag="e")
                    if dve_kc(qc, kc):
                        nc.vector.tensor_scalar(
                            out=e_sb.bitcast(i16),
                            in0=s_ps,
                            scalar1=_A_DVE,
                            scalar2=_B_DVE,
                            op0=mybir.AluOpType.mult,
                            op1=mybir.AluOpType.add,
                        )
                        nc.vector._custom_dve(
                            corr_op,
                            out=e_sb,
                            in0=e_sb,
                            in1=mask_t,
                            s0=_CORR_A,
                            s1=_CORR_Q2,
                            imm2=_CORR_C,
                        )
                    else:
                        nc.scalar.activation(e_sb, s_ps, Exp, scale=0.125)
                    pend.append((kc, e_sb))
                    if len(pend) > pend_depth:
                        emit_o(pend.pop(0))
                for blk in pend:
                    emit_o(blk)

                # reciprocals first (den_bc on the PE waits on them; the
                # copies only gate the next qc's attn@V), then park the raw
                # attention output in SBUF so the o_ps PSUM banks free up.
                # den at partition 64; h0 -> den2r partition 0, h1 -> 32.
                den2r = den2r_t[qc]
                with nc.allow_low_precision(reason="softmax denom broadcast"):
                    nc.vector.reciprocal(out=den2r[0:1, :], in_=o_ps[0][64:65, :])
                    nc.vector.reciprocal(out=den2r[32:33, :], in_=o_ps[1][64:65, :])
                nc.vector.tensor_copy(out=ot[qc][0:64, :], in_=o_ps[0][0:64, :])
                nc.vector.tensor_copy(out=ot[qc][64:128, :], in_=o_ps[1][0:64, :])
                deferred = make_outproj(qc, o_ps, den2r)

            # drain qc7's normalization + out-projection
            for kc in sorted(deferred):
                for fn in deferred[kc]:
                    fn()

    nc.finalize()
    return nc


def get_nc(n=_N, zero_bias=False, dve_split=True, **kw):
    key = (n, zero_bias, dve_split, tuple(sorted(kw.items())))
    if key not in _nc_cache:
        _nc_cache[key] = _build(n, zero_bias, dve_split, **kw)
    return _nc_cache[key]


def make_in_maps(Q, K, V, Wq, bq, Wk, bk, Wv, bv, Wo, bo, n=_N):
    """Per-core input dicts (host-side sharding / layout prep)."""
    bf = np.float16
    nqc = n // 512
    xts = {}
    for b in range(_B):
        d = {}
        for t, X in (("q", Q), ("k", K), ("v", V)):
            xt = X[b][:n].T.astype(bf)  # [512, n]
            d[f"xt{t}"] = np.ascontiguousarray(
                xt.reshape(4, 128, nqc, 512).transpose(2, 1, 0, 3).reshape(nqc, 128, 2048)
            )
        xts[b] = d
    in_maps = []
    for c in range(_NCORES):
        b, p = divmod(c, 4)
        off = p * 128
        m = dict(xts[b])
        for t, W, bias in (("q", Wq, bq), ("k", Wk, bk), ("v", Wv, bv)):
            m[f"w{t}"] = np.ascontiguousarray(
                W[:, off : off + 128]
                .reshape(4, 128, 128)
                .transpose(1, 0, 2)
                .reshape(128, 512)
                .astype(bf)
            )
            m[f"b{t}"] = np.ascontiguousarray(bias[off : off + 128].reshape(128, 1))
        m["wo"] = np.ascontiguousarray(Wo[off : off + 128].astype(bf))
        o2 = np.zeros((33, 128), np.float16)
        o2[0, 0:64] = 1.0
        o2[32, 64:128] = 1.0
        m["ones2"] = o2
        in_maps.append(m)
    return in_maps


def assemble(results, bo, n=_N):
    """Cross-core reduction: sum the (already normalized) per-head-pair
    partial outputs, add output bias, restore [B, N, D] layout."""
    out = np.empty((_B, n, _D), np.float32)
    for b in range(_B):
        acc = None
        for p in range(4):
            # y [4, nqc, 128, 512] -> [512, n]
            y = (
                results[4 * b + p]["y"]
                .astype(np.float32)
                .transpose(0, 2, 1, 3)
                .reshape(_D, n)
            )
            acc = y if acc is None else acc + y
        out[b] = acc.T + bo
    return out


def kernel(Q, K, V, Wq, bq, Wk, bk, Wv, bv, Wo, bo):
    from concourse import bass_utils

    args = [np.asarray(a, np.float32) for a in (Q, K, V, Wq, bq, Wk, bk, Wv, bv, Wo, bo)]
    Q, K, V, Wq, bq, Wk, bk, Wv, bv, Wo, bo = args
    zb = not (np.any(bq) or np.any(bk) or np.any(bv))
    nc = get_nc(zero_bias=zb)
    in_maps = make_in_maps(Q, K, V, Wq, bq, Wk, bk, Wv, bv, Wo, bo)
    res = bass_utils.run_bass_kernel_spmd(
        nc, in_maps, core_ids=list(range(_NCORES))
    )
    return assemble(res.results, bo)


# revision 39
# speedup vs baseline: 1.2240x; 1.1343x over previous
"""Multi-head attention (B=2, N=4096, D=512, H=8) on 8 trn2 NeuronCores.

Sharding: core c handles batch b = c//4 and head-pair p = c%4 (heads 2p,
2p+1).  Each core projects its batch's Q/K/V against its pair's weight
columns, computes transposed attention scores sT = K_h @ Q_h^T per
128-key chunk, applies exp((1/8)*sT) split between the ACT engine
(table exp) and the DVE (Schraudolph int16 bit-trick + 7-stage
quadratic correction op, sigma ~0.2%), multiplies by an augmented V
(extra ones column, M=65) so the softmax denominators fall out of the
same matmul, normalizes on-device (reciprocal of the denominator row,
broadcast across partitions via a tiny K=2 matmul, fused into the
PSUM->SBUF move of the attention output), and applies Wo with both
heads contracted in a single 128-deep matmul per output chunk.

Schedule notes (all aimed at keeping the PE busy 100% of the time so
it holds its top p-state):
  - only K0/V0/Q0 are projected up front; all other K/V/Q projections
    are woven into the attention stream of qc0 (K(nk) at kc=4nk-4,
    V(nk) at 4nk-3, V transposes at 4nk-2), so the PE is never
    DMA-starved.
  - scores for both heads of a key-chunk share one PSUM tile and one
    exp instruction ([128,1024]), halving exp instruction overhead.
  - attn@V matmuls run 4 key-chunks behind the scores matmuls so the
    PE never head-of-line blocks on exp results.
  - the end-of-qc normalization + out-projection are deferred into the
    next q-chunk's scores stream (den_bc at kc=1, o normalize at kc=1,
    out-proj pieces at kc=3,5,7,9) so the PE queue never waits on the
    DVE at a q-chunk boundary.
Head 1's V chunks use a [1|v] layout (ones column first) and its
attn@V output lands at partitions 63:128, so the normalizing
tensor_tensor multiplies are partition-aligned for both heads.

Device layouts (host pre-arranges):
  xt{q,k,v}: [8, 128, 2048]    tile (nk): X^T 4 x [128, 512] dc-chunks
  w{q,k,v}:  [128, 512]        w[p, dc*128+c] = W[dc*128+p, off+c]
  b{q,k,v}:  [128, 1]          pair slice of bias
  wo:        [128, 512]        wo[p, mt*128+c] = Wo[off+p, mt*128+c]
Output per core:
  y: [4, 8, 128, 512] bf16     tile (mt, qc): y^T[mt*128:+128, qc*512:+512],
                               already normalized and summed over the
                               core's two heads.
Final host step: out[b] = (sum_p y_p).T + bo
"""

import numpy as np

_B, _N, _D, _H, _DK = 2, 4096, 512, 8, 64
_NCORES = 8

_LN2 = float(np.log(2.0))
_A_DVE = 0.125 * 1024.0 / _LN2
_B_DVE = 15360.0
_CORR_A = -1.4763417585548537
_CORR_Q2 = 0.22711289921196798
_CORR_C = 0.9424678640725361

_nc_cache = {}
_exp_corr_op = None


def _get_exp_corr_op():
    """Register (once) the custom DVE op: out = ((u+C0)^2*C1 + C2) * Src0
    with u = bitwise_or(bitwise_and(Src0, Src1), 1.0f) — Src1 carries the
    fp32 mantissa mask 0x007FFFFF as a full-width tensor ([P,1] broadcast
    Src1 hangs the DVE on this runtime)."""
    global _exp_corr_op
    if _exp_corr_op is not None:
        return _exp_corr_op
    from concourse import dve_ops
    from concourse.dve_spec import (
        AluOp,
        Bin,
        C0,
        C1,
        C2,
        One,
        Spec,
        Src0,
        Src1,
        lower,
        sq,
    )
    from concourse.dve_uop import DveOpSpec

    name = "EXP16_CORR_ANT"
    for op in dve_ops.OPS:
        if op.name == name:
            _exp_corr_op = op
            return op

    u = Bin(AluOp.BITWISE_OR, Bin(AluOp.BITWISE_AND, Src0, Src1), One)
    body = (sq(u + C0) * C1 + C2) * Src0

    def _ref(in0, in1, s0, s1, imm2):
        b = np.asarray(in0, np.float32).view(np.uint32)
        m = np.asarray(in1, np.float32).view(np.uint32)
        uu = ((b & m) | np.uint32(0x3F800000)).view(np.float32)
        return ((uu + s0) ** 2 * s1 + imm2) * in0

    spec = Spec(body=body, reference=_ref)
    sha = {
        ver: DveOpSpec(name=name, uops=lower(spec, ver=ver)).sha(ver)
        for ver in ("v3", "v4")
    }
    op = dve_ops.DveOp(name, spec, subdim=False, uops_sha=sha)
    idx = len(dve_ops.OPS)
    dve_ops.OPS.append(op)
    dve_ops.CUSTOM_DVE_SPECS[name] = spec
    dve_ops._SUB_OPCODE_FOR_NAME[name] = dve_ops._CUSTOM_DVE_ROW_BASE + idx
    _exp_corr_op = op
    return op


def _build(
    n=_N,
    zero_bias=False,
    dve_split=True,
    mask16=False,
    pend_depth=7,
    # DVE exp key-chunks: middle kc only.  The first few kc of each qc
    # stay on ACT right after its boundary backlog clears (they feed the
    # first attn@V matmuls), and the last kc stay off the DVE so the
    # end-of-qc reciprocals/copies don't sit behind exp backlog — the
    # PE's in-order queue stalls on them at the boundary otherwise.
    dve_kcs_steady=(8, 11, 14, 17, 20, 23, 26, 29),
    dve_kcs_qc0=(7, 11, 15, 19, 23),
):
    import concourse.mybir as mybir
    import concourse.tile as tile
    from concourse import bacc
    from concourse.masks import make_identity

    f32 = mybir.dt.float32
    f32r = mybir.dt.float32r
    i16 = mybir.dt.int16
    i32 = mybir.dt.int32
    bf16 = mybir.dt.float16
    Exp = mybir.ActivationFunctionType.Exp
    Copy = mybir.ActivationFunctionType.Copy
    NKC = n // 128  # key chunks of 128 (PSUM partitions of sT)
    NQC = n // 512  # q chunks of 512

    corr_op = _get_exp_corr_op() if dve_split else None

    nc = bacc.Bacc(
        "TRN2", target_bir_lowering=False, debug=False, num_devices=_NCORES
    )

    xt = {
        t: nc.dram_tensor(f"xt{t}", [NQC, 128, 2048], bf16, kind="ExternalInput").ap()
        for t in "qkv"
    }
    w = {
        t: nc.dram_tensor(f"w{t}", [128, 512], bf16, kind="ExternalInput").ap()
        for t in "qkv"
    }
    bvec = {
        t: nc.dram_tensor(f"b{t}", [128, 1], f32, kind="ExternalInput").ap()
        for t in "qkv"
    }
    wo = nc.dram_tensor("wo", [128, 512], bf16, kind="ExternalInput").ap()
    ones2_in = nc.dram_tensor("ones2", [33, 128], f32, kind="ExternalInput").ap()
    y_out = nc.dram_tensor(
        "y", [4, NQC, 128, 512], bf16, kind="ExternalOutput"
    ).ap()

    _dve_steady = frozenset(dve_kcs_steady)
    _dve_qc0 = frozenset(dve_kcs_qc0)

    def dve_kc(qc, kc):
        if not dve_split:
            return False
        return kc in (_dve_qc0 if qc == 0 else _dve_steady)

    with tile.TileContext(nc) as tc:
        with (
            tc.tile_pool(name="consts", bufs=1) as consts,
            tc.tile_pool(name="xtp", bufs=6) as xtp,
            tc.tile_pool(name="persist", bufs=1) as persist,
            tc.tile_pool(name="ep", bufs=9) as ep,
            tc.tile_pool(name="ysbp", bufs=3) as ysbp,
            tc.tile_pool(name="psA", bufs=3, space="PSUM") as psA,
            tc.tile_pool(name="psB", bufs=1, space="PSUM") as psB,
        ):
            wsb, bsb = {}, {}
            for t in "qkv":
                wsb[t] = consts.tile([128, 512], bf16, name=f"w{t}sb", tag=f"w{t}sb")
            wosb = consts.tile([128, 512], bf16, name="wosb", tag="wosb")
            # startup: wk + xv0 ride the sync queue while xk0 + wv/wq/xq0
            # ride the scalar queue, so K0's operands land ~1us sooner and
            # V0/Q0 right behind (wo is only needed at qc0's out-proj).
            nc.sync.dma_start(out=wsb["k"], in_=w["k"])
            x0 = {}
            for t in ("k", "v", "q"):
                x0[t] = xtp.tile([128, 2048], bf16, name=f"x_{t}0", tag="xt")
            nc.scalar.dma_start(out=x0["k"][:, 0:1024], in_=xt["k"][0][:, 0:1024])
            nc.scalar.dma_start(out=x0["k"][:, 1024:2048], in_=xt["k"][0][:, 1024:2048])
            nc.sync.dma_start(out=x0["v"][:, 0:1024], in_=xt["v"][0][:, 0:1024])
            nc.sync.dma_start(out=x0["v"][:, 1024:2048], in_=xt["v"][0][:, 1024:2048])
            nc.scalar.dma_start(out=wsb["v"], in_=w["v"])
            nc.scalar.dma_start(out=wsb["q"], in_=w["q"])
            nc.scalar.dma_start(out=x0["q"][:, 0:1024], in_=xt["q"][0][:, 0:1024])
            nc.scalar.dma_start(out=x0["q"][:, 1024:2048], in_=xt["q"][0][:, 1024:2048])
            nc.scalar.dma_start(out=wosb, in_=wo)
            if not zero_bias:
                for t in "qkv":
                    bsb[t] = consts.tile([128, 1], f32, name=f"b{t}sb", tag=f"b{t}sb")
                    nc.sync.dma_start(out=bsb[t], in_=bvec[t])
            ident = consts.tile([128, 128], bf16, name="ident")
            make_identity(nc, ident)
            if dve_split:
                mdt = bf16 if mask16 else f32
                mask_t = consts.tile([128, 1024], mdt, name="mmask", tag="mmask")
                if mask16:
                    nc.gpsimd.memset(mask_t.bitcast(i16), 0x007F)
                else:
                    nc.gpsimd.memset(mask_t.bitcast(i32), 0x007FFFFF)
            # ones2: stationary for the denominator partition-broadcast
            # matmul: den_bc[j, :] = den2r[0 if j < 64 else 32, :].  K=33
            # with zero rows 1..31 because engine APs need 32-aligned
            # partition bases (so h1's reciprocal lands at partition 32).
            ones2 = consts.tile([33, 128], f32, name="ones2", tag="ones2")
            nc.scalar.dma_start(out=ones2, in_=ones2_in)

            qt_t = [
                persist.tile([128, 512], bf16, name=f"qt{i}", tag=f"qt{i}")
                for i in range(NQC)
            ]
            kt_t = [
                persist.tile([128, 512], bf16, name=f"kt{i}", tag=f"kt{i}")
                for i in range(NQC)
            ]
            vt_t = [
                persist.tile([128, 512], bf16, name=f"vt{i}", tag=f"vt{i}")
                for i in range(NQC)
            ]
            # augmented V chunks: [v|1] (ones col 64) for both heads; the
            # softmax denominator falls out of the attn@V matmul at
            # partition 64.
            vch = [
                [
                    persist.tile(
                        [128, 65], bf16, name=f"vch{h}_{c}", tag=f"vch{h}_{c}"
                    )
                    for c in range(NKC)
                ]
                for h in range(2)
            ]
            ot = [
                persist.tile([128, 512], bf16, name=f"ot{qc}", tag=f"ot{qc}")
                for qc in range(NQC)
            ]
            # den2r rows 1..31 stay 1.0 (not 0.0): reciprocal_approx_fast is
            # undefined at 0 and could leave inf/nan that would poison the
            # den_bc matmul through 0*inf.
            den2r_t = [
                persist.tile([33, 512], f32, name=f"dr{qc}", tag=f"dr{qc}")
                for qc in range(NQC)
            ]
            for qc in range(NQC):
                nc.gpsimd.memset(den2r_t[qc], 1.0)
            for c in range(NKC):
                nc.gpsimd.memset(vch[0][c][:, 64:65], 1.0)
                nc.gpsimd.memset(vch[1][c][:, 64:65], 1.0)

            def proj_dma(t, nk, dma_eng=None):
                xtile = xtp.tile([128, 2048], bf16, name=f"x_{t}{nk}", tag="xt")
                eng = dma_eng or nc.sync
                eng.dma_start(out=xtile[:, 0:1024], in_=xt[t][nk][:, 0:1024])
                eng.dma_start(out=xtile[:, 1024:2048], in_=xt[t][nk][:, 1024:2048])
                return xtile

            def proj(t, nk, dst, dma_eng=None, copy_eng="act", xtile=None):
                ppsum = psA.tile([128, 1024], f32, name=f"pp_{t}{nk}", tag="sblk")
                if xtile is None:
                    xtile = proj_dma(t, nk, dma_eng)
                pp = ppsum[:, 0:512]
                for dc in range(4):
                    nc.tensor.matmul(
                        pp,
                        wsb[t][:, dc * 128 : (dc + 1) * 128],
                        xtile[:, dc * 512 : (dc + 1) * 512],
                        start=(dc == 0),
                        stop=(dc == 3),
                    )
                if not zero_bias:
                    nc.vector.tensor_scalar_add(out=dst, in0=pp, scalar1=bsb[t])
                elif copy_eng == "act":
                    nc.scalar.activation(out=dst, in_=pp, func=Copy)
                else:
                    nc.vector.tensor_copy(out=dst, in_=pp)

            def vtrans(nk):
                # transpose the 4 key-chunks of V tile nk into per-head
                # augmented chunks
                pt = psA.tile([128, 1024], bf16, name=f"pt{nk}", tag="sblk")
                for j in range(4):
                    nc.tensor.transpose(
                        pt[:, j * 128 : (j + 1) * 128],
                        vt_t[nk][:, j * 128 : (j + 1) * 128],
                        ident,
                    )
                for j in range(4):
                    c = nk * 4 + j
                    nc.vector.tensor_copy(
                        out=vch[0][c][:, 0:64], in_=pt[:, j * 128 : j * 128 + 64]
                    )
                    nc.vector.tensor_copy(
                        out=vch[1][c][:, 0:64], in_=pt[:, j * 128 + 64 : j * 128 + 128]
                    )

            # ---- phase 1: K0 / V0 / Q0 only; the rest is woven into qc0 ----
            proj("k", 0, kt_t[0], copy_eng="act", xtile=x0["k"])
            proj("v", 0, vt_t[0], copy_eng="act", xtile=x0["v"])
            vtrans(0)
            proj("q", 0, qt_t[0], copy_eng="dve", xtile=x0["q"])

            def make_outproj(qc, o_ps, den2r):
                """Deferred normalization + out-projection pieces for qc,
                executed interleaved with qc+1's scores stream.  ot[qc]
                already holds the raw attention output (copied at the end
                of qc so the o_ps PSUM banks free early); here we broadcast
                the reciprocal denominators, scale ot in place, and run the
                out-projection."""
                den_bc_box = {}

                def den_bc_piece():
                    den_bc = psA.tile([128, 1024], f32, name=f"dbc{qc}", tag="sblk")
                    nc.tensor.matmul(
                        den_bc[:, 0:512],
                        ones2,
                        den2r,
                        start=True,
                        stop=True,
                        skip_group_check=True,
                    )
                    den_bc_box["t"] = den_bc

                def norm_piece():
                    den_bc = den_bc_box["t"]
                    nc.vector.tensor_mul(
                        out=ot[qc][0:64, :],
                        in0=ot[qc][0:64, :],
                        in1=den_bc[0:64, 0:512],
                    )
                    nc.vector.tensor_mul(
                        out=ot[qc][64:128, :],
                        in0=ot[qc][64:128, :],
                        in1=den_bc[64:128, 0:512],
                    )

                def mt_piece(mt):
                    y_ps = psA.tile([128, 1024], f32, name=f"y{qc}_{mt}", tag="sblk")
                    nc.tensor.matmul(
                        y_ps[:, 0:512],
                        wosb[:, mt * 128 : (mt + 1) * 128],
                        ot[qc],
                        start=True,
                        stop=True,
                        skip_group_check=True,
                    )
                    y_sb = ysbp.tile([128, 512], bf16, name=f"ysb{qc}_{mt}", tag="ysb")
                    if mt % 2 == 0:
                        nc.scalar.activation(out=y_sb, in_=y_ps[:, 0:512], func=Copy)
                    else:
                        nc.vector.tensor_copy(out=y_sb, in_=y_ps[:, 0:512])
                    nc.gpsimd.dma_start(out=y_out[mt, qc], in_=y_sb)

                return {
                    4: [den_bc_piece],
                    5: [norm_piece],
                    7: [lambda: mt_piece(0)],
                    9: [lambda: mt_piece(1)],
                    11: [lambda: mt_piece(2)],
                    13: [lambda: mt_piece(3)],
                }

            # ---- phase 2: attention, everything else woven in ----
            deferred = {}
            for qc in range(NQC):
                weave = {}
                xbox = {}

                def wv_dma(t, nk, eng, slot):
                    weave.setdefault(slot, []).append(
                        lambda: xbox.__setitem__((t, nk), proj_dma(t, nk, eng))
                    )

                def wv_mm(t, nk, dst, copy_eng, slot):
                    weave.setdefault(slot, []).append(
                        lambda: proj(
                            t, nk, dst, copy_eng=copy_eng, xtile=xbox[(t, nk)]
                        )
                    )

                if qc == 0:
                    # x-tile DMAs run two weave slots ahead of their matmuls
                    for nk in range(1, NQC):
                        wv_dma("k", nk, nc.sync, max(4 * nk - 6, 0))
                        wv_mm("k", nk, kt_t[nk], "act", 4 * nk - 4)
                        wv_dma("v", nk, nc.scalar, max(4 * nk - 5, 0))
                        wv_mm("v", nk, vt_t[nk], "act", 4 * nk - 3)
                        weave.setdefault(4 * nk - 2, []).append(
                            lambda nk=nk: vtrans(nk)
                        )
                    wv_dma("q", 1, nc.scalar, 12)
                    wv_mm("q", 1, qt_t[1], "dve", 14)
                    wv_dma("q", 2, nc.scalar, 16)
                    wv_mm("q", 2, qt_t[2], "dve", 18)
                elif qc + 2 < NQC:
                    wv_dma("q", qc + 2, nc.scalar, 14)
                    wv_mm("q", qc + 2, qt_t[qc + 2], "act", 16)

                o_ps = {
                    0: psB.tile([128, 512], f32, name=f"o0_{qc}", tag="oy0"),
                    1: psB.tile([128, 512], f32, name=f"o1_{qc}", tag="oy1"),
                }

                def emit_o(blk, qc=qc, o_ps=o_ps):
                    kc, e_sb = blk
                    nc.tensor.matmul(
                        o_ps[0][0:65, :],
                        vch[0][kc],
                        e_sb[:, 0:512],
                        start=(kc == 0),
                        stop=(kc == NKC - 1),
                        skip_group_check=True,
                    )
                    nc.tensor.matmul(
                        o_ps[1][0:65, :],
                        vch[1][kc],
                        e_sb[:, 512:1024],
                        start=(kc == 0),
                        stop=(kc == NKC - 1),
                        skip_group_check=True,
                    )

                pend = []
                for kc in range(NKC):
                    for fn in weave.get(kc, []):
                        fn()
                    for fn in deferred.get(kc, []):
                        fn()
                    s_ps = psA.tile([128, 1024], f32, name=f"s_{qc}_{kc}", tag="sblk")
                    for h in range(2):
                        hp = slice(h * 64, (h + 1) * 64)
                        nc.tensor.matmul(
                            s_ps[:, h * 512 : (h + 1) * 512],
                            kt_t[kc // 4][hp, (kc % 4) * 128 : (kc % 4 + 1) * 128],
                            qt_t[qc][hp, :],
                            start=True,
                            stop=True,
                            skip_group_check=True,
                        )
                    e_sb = ep.tile([128, 1024], bf16, name=f"e_{qc}_{kc}", tag="e")
                    if dve_kc(qc, kc):
                        nc.vector.tensor_scalar(
                            out=e_sb.bitcast(i16),
                            in0=s_ps,
                            scalar1=_A_DVE,
                            scalar2=_B_DVE,
                            op0=mybir.AluOpType.mult,
                            op1=mybir.AluOpType.add,
                        )
                        nc.vector._custom_dve(
                            corr_op,
                            out=e_sb,
                            in0=e_sb,
                            in1=mask_t,
                            s0=_CORR_A,
                            s1=_CORR_Q2,
                            imm2=_CORR_C,
                        )
                    else:
                        nc.scalar.activation(e_sb, s_ps, Exp, scale=0.125)
                    pend.append((kc, e_sb))
                    if len(pend) > pend_depth:
                        emit_o(pend.pop(0))
                for blk in pend:
                    emit_o(blk)

                # denominators first (den_bc on the PE waits on them): copy
                # the den rows (at o_ps partition 64) into den2r partitions
                # 0 / 32, take one fast reciprocal over the whole tile
                # in-place, then park the raw attention output in SBUF so
                # the o_ps PSUM banks free up for the next q-chunk.
                den2r = den2r_t[qc]
                nc.vector.tensor_copy(out=den2r[0:1, :], in_=o_ps[0][64:65, :])
                nc.vector.tensor_copy(out=den2r[32:33, :], in_=o_ps[1][64:65, :])
                nc.vector.reciprocal_approx_fast(out=den2r, in_=den2r)
                nc.vector.tensor_copy(out=ot[qc][0:64, :], in_=o_ps[0][0:64, :])
                nc.vector.tensor_copy(out=ot[qc][64:128, :], in_=o_ps[1][0:64, :])
                deferred = make_outproj(qc, o_ps, den2r)

            # drain qc7's normalization + out-projection
            for kc in sorted(deferred):
                for fn in deferred[kc]:
                    fn()

    nc.finalize()
    return nc


def get_nc(n=_N, zero_bias=False, dve_split=True, **kw):
    key = (n, zero_bias, dve_split, tuple(sorted(kw.items())))
    if key not in _nc_cache:
        _nc_cache[key] = _build(n, zero_bias, dve_split, **kw)
    return _nc_cache[key]


def make_in_maps(Q, K, V, Wq, bq, Wk, bk, Wv, bv, Wo, bo, n=_N):
    """Per-core input dicts (host-side sharding / layout prep)."""
    bf = np.float16
    nqc = n // 512
    xts = {}
    for b in range(_B):
        d = {}
        for t, X in (("q", Q), ("k", K), ("v", V)):
            xt = X[b][:n].T.astype(bf)  # [512, n]
            d[f"xt{t}"] = np.ascontiguousarray(
                xt.reshape(4, 128, nqc, 512).transpose(2, 1, 0, 3).reshape(nqc, 128, 2048)
            )
        xts[b] = d
    in_maps = []
    for c in range(_NCORES):
        b, p = divmod(c, 4)
        off = p * 128
        m = dict(xts[b])
        for t, W, bias in (("q", Wq, bq), ("k", Wk, bk), ("v", Wv, bv)):
            m[f"w{t}"] = np.ascontiguousarray(
                W[:, off : off + 128]
                .reshape(4, 128, 128)
                .transpose(1, 0, 2)
                .reshape(128, 512)
                .astype(bf)
            )
            m[f"b{t}"] = np.ascontiguousarray(bias[off : off + 128].reshape(128, 1))
        m["wo"] = np.ascontiguousarray(Wo[off : off + 128].astype(bf))
        o2 = np.zeros((33, 128), np.float32)
        o2[0, 0:64] = 1.0
        o2[32, 64:128] = 1.0
        m["ones2"] = o2
        in_maps.append(m)
    return in_maps


def assemble(results, bo, n=_N):
    """Cross-core reduction: sum the (already normalized) per-head-pair
    partial outputs, add output bias, restore [B, N, D] layout."""
    out = np.empty((_B, n, _D), np.float32)
    for b in range(_B):
        acc = None
        for p in range(4):
            # y [4, nqc, 128, 512] -> [512, n]
            y = (
                results[4 * b + p]["y"]
                .astype(np.float32)
                .transpose(0, 2, 1, 3)
                .reshape(_D, n)
            )
            acc = y if acc is None else acc + y
        out[b] = acc.T + bo
    return out


def kernel(Q, K, V, Wq, bq, Wk, bk, Wv, bv, Wo, bo):
    from concourse import bass_utils

    args = [np.asarray(a, np.float32) for a in (Q, K, V, Wq, bq, Wk, bk, Wv, bv, Wo, bo)]
    Q, K, V, Wq, bq, Wk, bk, Wv, bv, Wo, bo = args
    zb = not (np.any(bq) or np.any(bk) or np.any(bv))
    nc = get_nc(zero_bias=zb)
    in_maps = make_in_maps(Q, K, V, Wq, bq, Wk, bk, Wv, bv, Wo, bo)
    res = bass_utils.run_bass_kernel_spmd(
        nc, in_maps, core_ids=list(range(_NCORES))
    )
    return assemble(res.results, bo)
